# revision 90
# baseline (speedup 1.0000x reference)
"""AttentionLePE distributed Trainium2 kernel (fp8-DoubleRow + 3-engine softmax).

Data-parallel over batch (8 batch elements -> 8 cores, no collectives).
Cost-model-driven design (TimelineSim charges matmuls by output free-size
x cycles/row, with fp8 DoubleRow at 0.5 cycles/row):

  - qkv matmuls: fp8e4 DoubleRow (contraction 512 = 2 chunks of 2x128)
  - scores S^T:  fp8e4 DoubleRow with a zeros second slice (contraction 32)
  - softmax exp: ScalarE exact exp -> fp8e5 for heads j0/j1, DVE
    Schraudolph bitcast-exp for j2/j3 (i = uint8(x*a+b) reinterpreted as
    fp8e5m2; no clamp needed since e5m2 spans exp(+-10 sigma) logits)
  - p@v + denominator: fp8 DoubleRow over 256-token pT pairs; v/ones are
    zero-padded to 128-row stationary operands because the ISA requires
    DR matmul outputs to start at PSUM partition 0
  - LePE 5x5 depthwise conv: 24 taps as PE diag matmuls (bf16) into a
    PSUM accumulator evacuated by ScalarE, 1 tap applied by DVE into aoT
  - proj: bf16, output shipped bf16 and cast to f32 on host

Platform rules this design is shaped by: every PSUM tile serializes ALL
its reader instructions (even disjoint cross-engine reads) -> one exp
instruction per score tile; GPSIMD/Pool cannot access PSUM and has no
tensor_scalar -> Act+DVE carry all PSUM evacuations.

PSUM (8 banks): st01 [128,1024] (ScalarE exp) + st23 [128,1024] (DVE
exp) + transient [128,512] + out/den/lepe accumulators.
"""

import numpy as np
import ml_dtypes

B, Hs, Ws, C = 8, 32, 32, 512
N = Hs * Ws          # 1024 tokens
HEADS = 16
HD = C // HEADS      # 32
KS = 5
SCALE = float(HD) ** -0.5
NCORES = 8

_BF16 = ml_dtypes.bfloat16
_F8E4 = ml_dtypes.float8_e4m3
_F8E5 = ml_dtypes.float8_e5m2

# Schraudolph constants for exp(x*SCALE - MARGIN) -> fp8e5m2 byte via
# uint8(trunc(raw * SCH_A + SCH_B)).  MARGIN also applied to the exact
# ScalarE exp path so all engines produce consistently-scaled P tilde.
MARGIN = 2.0
_SCH_SLOPE = 5.7707801635558535      # 4*log2(e)
_SCH_BIAS = 48.70843967288829        # tuned offline, rms rel err 5.5%
SCH_A = SCALE * _SCH_SLOPE
SCH_B = _SCH_BIAS - _SCH_SLOPE * MARGIN

# LePE tap assignment (dh, dw).  GPSIMD/Pool cannot access PSUM and its
# ISA has no tensor_scalar, so taps live on PE (diag matmuls into a PSUM
# accumulator) with a couple on DVE applied directly into aoT after the
# ScalarE lp-copy.  Act+DVE carry all PSUM reads (exp, evacs, recip).
TP_PE = [(-1, -1), (-1, 0), (-1, 1), (0, -1), (0, 1), (1, -1), (1, 0), (1, 1),
         (0, -2), (0, 2), (-2, 0), (2, 0), (-2, -2), (-2, 2), (2, -2), (2, 2),
         (-2, -1), (-1, -2), (-1, 2), (1, -2), (1, 2), (-2, 1), (2, -1),
         (0, 0)]
TP_DVE = [(2, 1)]
ALL_TAPS = [(dh, dw) for dh in range(-2, 3) for dw in range(-2, 3)]

# Exp split: scores j0+j1 land in one 2-bank PSUM tile read by a single
# ScalarE exact-exp instruction; j2+j3 in another read by a single DVE
# Schraudolph instruction.  One reader per PSUM tile (the tile framework
# serializes all readers of a PSUM tile, even disjoint cross-engine
# reads), merged so the per-instruction overhead is paid once.

LAST_EXEC_TIME_NS = None
LAST_RESULTS = None


def _build_graph():
    import os as _os
    LOOP = int(_os.environ.get("ATTN_LEPE_LOOP", "1"))
    import concourse.bacc as bacc
    import concourse.mybir as mybir
    import concourse.tile as tile

    dt = mybir.dt
    AF = mybir.ActivationFunctionType
    ALU = mybir.AluOpType
    DR = mybir.MatmulPerfMode.DoubleRow

    nc = bacc.Bacc(
        "TRN2",
        target_bir_lowering=False,
        debug=False,
        enable_asserts=False,
        num_devices=NCORES,
    )

    # ---- dram tensors ----
    xT_d = nc.dram_tensor("xT", [C, N], dt.bfloat16, kind="ExternalInput")
    xf8_d = nc.dram_tensor("xf8", [128, 4 * N], dt.float8e4, kind="ExternalInput")
    wqkf8_d = nc.dram_tensor("wqkf8", [128, 4 * 1024], dt.float8e4,
                             kind="ExternalInput")
    wvf8_d = nc.dram_tensor("wvf8", [128, 4 * 512], dt.float8e4,
                            kind="ExternalInput")
    wpT_d = nc.dram_tensor("wpT", [C, C], dt.bfloat16, kind="ExternalInput")
    onesdr_d = nc.dram_tensor("onesdr", [128, 1024], dt.float8e4,
                              kind="ExternalInput")
    NTP = len(TP_PE)
    lepe_d = nc.dram_tensor("lepe", [128, NTP * 4 * 128], dt.bfloat16,
                            kind="ExternalInput")
    lepec_d = nc.dram_tensor("lepec", [128, KS * KS * 4], dt.float32,
                             kind="ExternalInput")
    beff_d = nc.dram_tensor("beff", [128, 4], dt.float32, kind="ExternalInput")
    out_d = nc.dram_tensor("out", [C, N], dt.bfloat16, kind="ExternalOutput")

    NT = N // 128   # 8 token tiles
    CT = C // 128   # 4 channel groups
    NC2 = N // 512  # 2 q-chunks

    with tile.TileContext(nc) as tc:
        with (
            tc.tile_pool(name="persist", bufs=1) as persist,
            tc.tile_pool(name="pT", bufs=4) as pT_pool,
            tc.tile_pool(name="dr", bufs=4) as dr_pool,
            tc.tile_pool(name="acc", bufs=4) as acc_pool,
            tc.tile_pool(name="ps_A01", bufs=1, space="PSUM") as ps_A01,
            tc.tile_pool(name="ps_B23", bufs=1, space="PSUM") as ps_B23,
            tc.tile_pool(name="ps_T", bufs=1, space="PSUM") as ps_T,
            tc.tile_pool(name="ps_out", bufs=1, space="PSUM") as ps_out,
            tc.tile_pool(name="ps_den", bufs=1, space="PSUM") as ps_den,
            tc.tile_pool(name="ps_lepe", bufs=1, space="PSUM") as ps_lepe,
        ):
            # ---- persistent SBUF loads (critical first: xf8 + wqk) ----
            xf8 = persist.tile([128, 4 * N], dt.float8e4, tag="xf8", name="xf8")
            nc.sync.dma_start(xf8[:], xf8_d[:, :])
            # wqkf8 layout [p, f(8), cblk(4), 128]; f=4 (k of g0) and f=0
            # (q of g0) are needed by the head qk groups -> tiny DMAs first
            wqkf8 = persist.tile([128, 4096], dt.float8e4, tag="wqkf8",
                                 name="wqkf8")
            nc.sync.dma_start(wqkf8[:, 2048:2560], wqkf8_d[:, 2048:2560])
            nc.sync.dma_start(wqkf8[:, 0:512], wqkf8_d[:, 0:512])
            nc.sync.dma_start(wqkf8[:, 512:2048], wqkf8_d[:, 512:2048])
            nc.sync.dma_start(wqkf8[:, 2560:4096], wqkf8_d[:, 2560:4096])
            wvf8 = persist.tile([128, 2048], dt.float8e4, tag="wvf8", name="wvf8")
            nc.sync.dma_start(wvf8[:], wvf8_d[:, :])
            # [p, 2, 4, 128]: variant j = [z(32j) | ones32 | z(96-32j)] so
            # DoubleRow p@v/den outputs are full 128-partition, base-0 (the
            # ISA requires DR matmul dst to start at PSUM partition 0)
            onesdr = persist.tile([128, 1024], dt.float8e4, tag="onesdr",
                                  name="onesdr")
            nc.sync.dma_start(onesdr[:], onesdr_d[:, :])
            # xT (bf16, for LePE) — needed from step ~1 on
            xT = []
            for g in range(CT):
                t = persist.tile([128, N], dt.bfloat16, tag=f"xT{g}", name=f"xT{g}")
                nc.sync.dma_start(t[:], xT_d[g * 128:(g + 1) * 128, :])
                xT.append(t)
            # non-critical loads, deferred past head kickoff
            wpT = [persist.tile([128, C], dt.bfloat16, tag=f"wpT{g}",
                                name=f"wpT{g}") for g in range(CT)]
            lepe_w = persist.tile([128, NTP * 4 * 128], dt.bfloat16, tag="lepe",
                                  name="lepe_w")
            beff_sb = persist.tile([128, 4], dt.float32, tag="beff", name="beff_sb")
            lepec_sb = persist.tile([128, KS * KS * 4], dt.float32, tag="lepec",
                                    name="lepec_sb")

            def load_noncritical():
                nch = (NTP * 4 * 128) // 2048
                for h in range(nch):
                    nc.sync.dma_start(
                        lepe_w[:, h * 2048:(h + 1) * 2048],
                        lepe_d[:, h * 2048:(h + 1) * 2048])
                nc.sync.dma_start(lepec_sb[:], lepec_d[:, :])
                for g in range(CT):
                    nc.sync.dma_start(wpT[g][:], wpT_d[g * 128:(g + 1) * 128, :])
                nc.sync.dma_start(beff_sb[:], beff_d[:, :])

            # persistent fp8 q/k tiles with zeros strip (cols 1024:2048)
            qf8 = [persist.tile([128, 2048], dt.float8e4, tag=f"qf8{g}",
                                name=f"qf8{g}") for g in range(4)]
            kf8 = [persist.tile([128, 2048], dt.float8e4, tag=f"kf8{g}",
                                name=f"kf8{g}") for g in range(4)]
            # v fp8 pair tiles [128, 2, 16 blocks, 128]: block (j,g) holds
            # [v_h(32) | zeros96] for head h=4g+j.  The p@v lhsT slice starts
            # 32j cols BEFORE the block so v lands on out partitions 32j --
            # the preceding cols are the previous block's zero tail.
            vf8 = [persist.tile([128, 4096], dt.float8e4, tag=f"vf8{p}",
                                name=f"vf8{p}") for p in range(4)]

            mbias = persist.tile([128, 1], dt.float32, tag="mbias", name="mbias")
            nc.gpsimd.memset(mbias[:], -MARGIN)

            xf8_3 = xf8[:].rearrange("p (c n) -> p c n", c=4)
            wqk_4 = wqkf8[:].rearrange("p (f c n) -> p f c n", f=8, c=4)
            wv_3 = wvf8[:].rearrange("p (c n) -> p c n", c=4)
            ones_3 = onesdr[:].rearrange("p (i n) -> p i n", i=2)

            def _zero_vf8_tiles():
                for g in range(4):
                    nc.gpsimd.memset(kf8[g][:, 1024:2048], 0.0)
                    nc.gpsimd.memset(qf8[g][:, 1024:2048], 0.0)
                    nc.gpsimd.memset(vf8[g][:], 0.0)

            for _it in range(LOOP):
                if _it == 0:
                    # zero strips + vf8 padding once (values persist).  Pool
                    # is idle at startup; g=0 first since the first score
                    # matmul reads its strips at ~7.5us.
                    _zero_vf8_tiles()

                aoT = [persist.tile([128, N], dt.bfloat16, tag=f"aoT{g}",
                                    name=f"aoT{g}") for g in range(4)]
                yT_sb = [persist.tile([128, N], dt.bfloat16, tag=f"yT{co}",
                                      name=f"yT{co}") for co in range(CT)]
                x3 = [xT[g][:].rearrange("p (h w) -> p h w", w=Ws)
                      for g in range(CT)]

                # ---------- qkv producers ----------
                def emit_qk(f, nc2):
                    # f in 0..7: 0-3 = q group g, 4-7 = k group g-4
                    dst = qf8[f] if f < 4 else kf8[f - 4]
                    ncs = slice(nc2 * 512, (nc2 + 1) * 512)
                    ps = ps_T.tile([128, 512], dt.float32, tag="T",
                                   name=f"qkps{f}_{nc2}")
                    for ch in range(2):
                        nc.tensor.matmul(
                            out=ps[:],
                            lhsT=wqk_4[:, f, 2 * ch:2 * ch + 2, :],
                            rhs=xf8_3[:, 2 * ch:2 * ch + 2, ncs],
                            start=(ch == 0), stop=(ch == 1),
                            perf_mode=DR,
                        )
                    nc.scalar.copy(dst[:, ncs], ps[:])

                def emit_v(m):
                    ps = ps_T.tile([128, 512], dt.float32, tag="T",
                                   name=f"vps{m}")
                    for ch in range(2):
                        nc.tensor.matmul(
                            out=ps[:],
                            lhsT=xf8_3[:, 2 * ch:2 * ch + 2,
                                       m * 128:(m + 1) * 128],
                            rhs=wv_3[:, 2 * ch:2 * ch + 2, :],
                            start=(ch == 0), stop=(ch == 1),
                            perf_mode=DR,
                        )
                    vp = vf8[m // 2][:].rearrange(
                        "p (i j g c) -> p i j g c", i=2, j=4, g=4)
                    ps_h = ps[:].rearrange("p (g j c) -> p j g c", g=4, j=4)
                    nc.scalar.copy(vp[:, m % 2, :, :, 0:32], ps_h)

                def emit_proj(co, nc2, rev=True, pool=None, tag=None,
                              w=512):
                    ncs = slice(nc2 * 512, (nc2 + 1) * 512)
                    y_full = (pool or ps_T).tile(
                        [128, w], dt.float32, tag=tag or "T",
                        name=f"yps{co}_{nc2}")
                    y_ps = y_full if w == 512 else y_full[:, 0:512]
                    # default: contract c = 3..0 so the group cannot start
                    # (and wedge the transient bank) until all inputs are
                    # ready; the tail uses rev=False to pre-accumulate
                    # c=0..2 while the last sweep still runs
                    order = range(CT - 1, -1, -1) if rev else range(CT)
                    for c in order:
                        nc.tensor.matmul(
                            out=y_ps[:],
                            lhsT=wpT[c][:, co * 128:(co + 1) * 128],
                            rhs=aoT[c][:, ncs],
                            start=(c == (CT - 1 if rev else 0)),
                            stop=(c == (0 if rev else CT - 1)),
                        )
                    nc.scalar.add(
                        yT_sb[co][:, ncs], y_ps[:], beff_sb[:, co:co + 1])
                    nc.sync.dma_start(out_d[co * 128:(co + 1) * 128, ncs],
                                      yT_sb[co][:, ncs])

                # ---------- LePE tap emitters ----------
                def lepe_pe_mms(g, hb, lp3):
                    mms = []
                    for pi, (dh, dw) in enumerate(TP_PE):
                        r0, r1 = max(0, -dh), Hs - max(0, dh)
                        w0, w1 = max(0, -dw), Ws - max(0, dw)
                        hr0, hr1 = max(r0, hb * 16), min(r1, hb * 16 + 16)
                        if hr0 >= hr1:
                            continue
                        diag = lepe_w[:, (pi * 4 + g) * 128:(pi * 4 + g + 1) * 128]

                        def mm(pi=pi, hr0=hr0, hr1=hr1, w0=w0, w1=w1,
                               dh=dh, dw=dw, diag=diag, lp3=lp3, g=g, hb=hb):
                            nc.tensor.matmul(
                                out=lp3[:, hr0 - hb * 16:hr1 - hb * 16, w0:w1],
                                lhsT=diag,
                                rhs=x3[g][:, hr0 + dh:hr1 + dh, w0 + dw:w1 + dw],
                                start=(pi == 0), stop=(pi == len(TP_PE) - 1),
                                skip_group_check=True,
                            )
                        mms.append(mm)
                    return mms

                def lepe_eng_units(g, hb, acc3, taps, engine):
                    units = []
                    for idx, (dh, dw) in enumerate(taps):
                        ti = ALL_TAPS.index((dh, dw))
                        r0, r1 = max(0, -dh), Hs - max(0, dh)
                        w0, w1 = max(0, -dw), Ws - max(0, dw)
                        hr0, hr1 = max(r0, hb * 16), min(r1, hb * 16 + 16)
                        if hr0 >= hr1:
                            continue

                        def u(idx=idx, ti=ti, hr0=hr0, hr1=hr1, w0=w0, w1=w1,
                              dh=dh, dw=dw, acc3=acc3, g=g, hb=hb,
                              engine=engine):
                            dst = acc3[:, hr0 - hb * 16:hr1 - hb * 16, w0:w1]
                            src = x3[g][:, hr0 + dh:hr1 + dh, w0 + dw:w1 + dw]
                            sc = lepec_sb[:, ti * 4 + g:ti * 4 + g + 1]
                            nc.vector.affine_then_add(
                                out=dst, in0=src, in1=dst,
                                scale=sc, bias=0.0)
                        units.append(u)
                    return units

                # ---------- head ----------
                emit_qk(4, 0)
                emit_qk(4, 1)
                emit_qk(0, 0)
                if _it == 0:
                    load_noncritical()

                fillers = [(1, lambda: emit_v(0)), (1, lambda: emit_v(1))]
                for m in range(2, NT):
                    fillers.append((m, lambda m=m: emit_v(m)))
                qk_sched = [((5, 0), 5), ((5, 1), 6), ((1, 0), 7),
                            ((6, 0), 13), ((6, 1), 14), ((2, 0), 15),
                            ((7, 0), 21), ((7, 1), 22), ((3, 0), 23),
                            ((0, 1), 30), ((1, 1), 38), ((2, 1), 46),
                            ((3, 1), 54)]
                for (f, nc2), dl in qk_sched:
                    fillers.append((dl, lambda f=f, nc2=nc2: emit_qk(f, nc2)))
                fillers.sort(key=lambda x: x[0])
                late = [(co, 0) for co in range(CT)]

                sweeps = [(nc2, g) for nc2 in range(NC2) for g in range(4)]
                steps = [(nc2, g, m) for (nc2, g) in sweeps for m in range(NT)]
                state = {}

                def sweep_tiles(nc2, g):
                    out_ps = ps_out.tile([128, 512], dt.float32, tag="out",
                                         name=f"outps{g}_{nc2}")
                    den_ps = ps_den.tile([128, 512], dt.float32, tag="den",
                                         name=f"denps{g}_{nc2}")
                    lp_ps = ps_lepe.tile([128, 512], dt.float32, tag="lp",
                                         name=f"lp{g}_{nc2}")
                    lp3 = lp_ps[:].rearrange("p (h w) -> p h w", w=Ws)
                    ao3 = aoT[g][:, nc2 * 512:(nc2 + 1) * 512].rearrange(
                        "p (h w) -> p h w", w=Ws)
                    return dict(
                        out_ps=out_ps, den_ps=den_ps, lp_ps=lp_ps,
                        pe=lepe_pe_mms(g, nc2, lp3),
                        dve_post=lepe_eng_units(g, nc2, ao3, TP_DVE, "dve"),
                        pT={},
                    )

                def emit_sT(nc2, g, m, pTr):
                    ncs = slice(nc2 * 512, (nc2 + 1) * 512)
                    q3 = qf8[g][:].rearrange("p (i n) -> p i n", i=2)
                    k3 = kf8[g][:].rearrange("p (i n) -> p i n", i=2)
                    ph = m % 2

                    def score_mm(j, out_ap):
                        nc.tensor.matmul(
                            out=out_ap,
                            lhsT=k3[j * 32:(j + 1) * 32, :,
                                    m * 128:(m + 1) * 128],
                            rhs=q3[j * 32:(j + 1) * 32, :, ncs],
                            start=True, stop=True,
                            perf_mode=DR,
                            tile_position=(j * 32, 0),
                        )

                    st23 = ps_B23.tile([128, 1024], dt.float32, tag="B23",
                                       name=f"st23_{g}_{nc2}_{m}")
                    score_mm(2, st23[:, 0:512])
                    score_mm(3, st23[:, 512:1024])
                    st01 = ps_A01.tile([128, 1024], dt.float32, tag="A01",
                                       name=f"st01_{g}_{nc2}_{m}")
                    score_mm(0, st01[:, 0:512])
                    score_mm(1, st01[:, 512:1024])
                    # one exp instruction per PSUM tile, one engine each
                    nc.vector.tensor_scalar(
                        out=pTr[:, ph, 1024:2048].bitcast(dt.uint8),
                        in0=st23[:],
                        scalar1=SCH_A, scalar2=SCH_B,
                        op0=ALU.mult, op1=ALU.add)
                    nc.scalar.activation(
                        pTr[:, ph, 0:1024], st01[:],
                        AF.Exp, scale=SCALE, bias=mbias[:])

                def emit_pv(nc2, g, pair, pTr):
                    st = state[(nc2, g)]
                    v3 = vf8[pair][:].rearrange("p (i n) -> p i n", i=2)
                    for j in range(4):
                        # block (j, g) at col (j*4+g)*128; slice shifted back
                        # 32j cols so v lands on out partitions 32j..32j+32
                        st0 = (j * 4 + g) * 128 - 32 * j
                        nc.tensor.matmul(
                            out=st["out_ps"][:],
                            lhsT=v3[:, :, st0:st0 + 128],
                            rhs=pTr[:, :, j * 512:(j + 1) * 512],
                            start=(pair == 0 and j == 0), stop=(pair == 3),
                            perf_mode=DR,
                            tile_position=(0, 0),
                            skip_group_check=True,
                        )
                        nc.tensor.matmul(
                            out=st["den_ps"][:],
                            lhsT=ones_3[:, :, j * 128:(j + 1) * 128],
                            rhs=pTr[:, :, j * 512:(j + 1) * 512],
                            start=(pair == 0 and j == 0), stop=(pair == 3),
                            perf_mode=DR,
                            tile_position=(0, 0),
                            skip_group_check=True,
                        )

                def emit_epilogue(nc2, g):
                    st = state.pop((nc2, g))
                    ncs = slice(nc2 * 512, (nc2 + 1) * 512)
                    # LePE into aoT: ScalarE evacuates the PSUM part, the
                    # DVE leftover taps apply on top
                    nc.scalar.copy(aoT[g][:, ncs], st["lp_ps"][:])
                    for u in st["dve_post"]:
                        u()
                    drec = dr_pool.tile([128, 512], dt.float32, tag="drec",
                                        name="drec")
                    nc.vector.reciprocal_approx_fast(out=drec[:],
                                                     in_=st["den_ps"][:])
                    tmp = dr_pool.tile([128, 512], dt.float32, tag="ntmp",
                                       name="ntmp")
                    nc.vector.tensor_mul(tmp[:], st["out_ps"][:], drec[:])
                    nc.gpsimd.tensor_tensor(
                        out=aoT[g][:, ncs], in0=tmp[:],
                        in1=aoT[g][:, ncs], op=ALU.add)

                # ---------- 64-step pipeline ----------
                prev_sweep = None
                for i, (nc2, g, m) in enumerate(steps):
                    while fillers and fillers[0][0] <= i:
                        fillers.pop(0)[1]()
                    if m == 0:
                        state[(nc2, g)] = sweep_tiles(nc2, g)
                    st = state[(nc2, g)]
                    pair = m // 2
                    if m % 2 == 0:
                        t = pT_pool.tile([128, 4096], dt.float8e5, tag="pT",
                                         name=f"pT{g}_{nc2}_{pair}")
                        st["pT"][pair] = t[:].rearrange("p (i n) -> p i n", i=2)
                    emit_sT(nc2, g, m, st["pT"][pair])
                    if m in (2, 4, 6):
                        emit_pv(nc2, g, pair - 1, st["pT"].pop(pair - 1))
                    if m == 0 and prev_sweep is not None:
                        pnc2, pg = prev_sweep
                        pst = state[(pnc2, pg)]
                        emit_pv(pnc2, pg, 3, pst["pT"].pop(3))
                        emit_epilogue(pnc2, pg)
                    # LePE taps: PE dribbled, all emitted by m=6 so the lp
                    # accumulation closes before the epilogue needs it
                    npe = 0 if m == 0 else (5 if m < NT - 2 else len(st["pe"]))
                    for _ in range(min(npe, len(st["pe"]))):
                        st["pe"].pop(0)()
                    if late and i >= 34 and i % 3 == 0:
                        emit_proj(*late.pop(0))
                    prev_sweep = (nc2, g)

                # tail
                pnc2, pg = prev_sweep
                pst = state[(pnc2, pg)]
                emit_pv(pnc2, pg, 3, pst["pT"].pop(3))
                emit_epilogue(pnc2, pg)
                while late:
                    emit_proj(*late.pop(0))
                # tail: pre-accumulate c=0..2 against already-final aoT
                # halves while the last sweep drains; co1/co2 borrow the
                # score pools so all four groups pre-accumulate in parallel
                emit_proj(0, 1, rev=False)
                emit_proj(1, 1, rev=False, pool=ps_A01, tag="A01", w=1024)
                emit_proj(2, 1, rev=False, pool=ps_B23, tag="B23", w=1024)
                emit_proj(3, 1, rev=False)

    nc.finalize()
    return nc


_GRAPH = None


def kernel(x, w_qkv, w_proj, b_proj, w_lepe, b_lepe, _trace=False):
    global _GRAPH, LAST_EXEC_TIME_NS, LAST_RESULTS
    from concourse.bass_utils import run_bass_kernel_spmd

    x = np.asarray(x, dtype=np.float32)
    w_qkv = np.asarray(w_qkv, dtype=np.float32)
    w_proj = np.asarray(w_proj, dtype=np.float32)
    b_proj = np.asarray(b_proj, dtype=np.float32)
    w_lepe = np.asarray(w_lepe, dtype=np.float32)
    b_lepe = np.asarray(b_lepe, dtype=np.float32)

    # fp8 weight layouts: [p, cblk, col] with c_in = cblk*128 + p
    wqk = w_qkv[:2 * C, :]                       # [1024, 512]
    # [p, f(8), cblk(4), 128]: element = w_qkv[f*128+n, cb*128+p]
    wqkf8 = np.ascontiguousarray(
        wqk.T.reshape(4, 128, 8, 128).transpose(1, 2, 0, 3).reshape(128, 4096)
    ).astype(_F8E4)
    wv = w_qkv[2 * C:, :]                        # [512, 512]
    wvf8 = np.ascontiguousarray(
        wv.T.reshape(4, 128, 512).transpose(1, 0, 2).reshape(128, 2048)
    ).astype(_F8E4)
    wpT = np.ascontiguousarray(w_proj.T).astype(_BF16)
    beff = (w_proj @ b_lepe + b_proj).astype(np.float32)
    beff_t = np.ascontiguousarray(beff.reshape(4, 128).T)
    onesdr = np.zeros((128, 1024), dtype=_F8E4)
    for i in range(2):
        for j in range(4):
            c0 = i * 512 + j * 128 + 32 * j
            onesdr[:, c0:c0 + 32] = 1.0

    wl = w_lepe.reshape(C, KS, KS)
    lepe_flat = np.zeros((128, len(TP_PE) * 4 * 128), dtype=_BF16)
    for pi, (dh, dw) in enumerate(TP_PE):
        for g in range(4):
            col0 = (pi * 4 + g) * 128
            wcol = wl[g * 128:(g + 1) * 128, dh + 2, dw + 2].astype(_BF16)
            lepe_flat[np.arange(128), col0 + np.arange(128)] = wcol
    lepe_col = np.zeros((128, KS * KS * 4), dtype=np.float32)
    for ti, (dh, dw) in enumerate(ALL_TAPS):
        for g in range(4):
            lepe_col[:, ti * 4 + g] = wl[g * 128:(g + 1) * 128, dh + 2, dw + 2]

    in_maps = []
    for b in range(NCORES):
        xb = x[b].reshape(N, C)                  # [1024, 512]
        xT = np.ascontiguousarray(xb.T).astype(_BF16)
        xf8 = np.ascontiguousarray(
            xb.T.reshape(4, 128, N).transpose(1, 0, 2).reshape(128, 4 * N)
        ).astype(_F8E4)
        in_maps.append({
            "xT": xT,
            "xf8": xf8,
            "wqkf8": wqkf8,
            "wvf8": wvf8,
            "wpT": wpT,
            "onesdr": onesdr,
            "lepe": lepe_flat,
            "lepec": lepe_col,
            "beff": beff_t,
        })

    if _GRAPH is None:
        _GRAPH = _build_graph()

    res = run_bass_kernel_spmd(_GRAPH, in_maps, list(range(NCORES)), trace=_trace)
    LAST_EXEC_TIME_NS = res.exec_time_ns
    LAST_RESULTS = res

    out = np.empty((B, Hs, Ws, C), dtype=np.float32)
    for b in range(NCORES):
        yT = np.asarray(res.results[b]["out"], dtype=np.float32)  # [C, N]
        out[b] = yT.T.reshape(Hs, Ws, C)
    return out
